# revision 1
# baseline (speedup 1.0000x reference)
"""Trainium2 Bass kernel for nn_GATv2_23278722744604.

2-layer GATv2 (N=50000 nodes, E=800000 edges, 128 feats, 4 heads x 32).
Sharding: destination-node blocks across 8 NeuronCores; edges routed to the
owner of their dst node; per-edge source features fetched by dma_gather from a
replicated projected table (bf16); segment softmax + aggregation via one-hot
matmul into PSUM. Two launches (one per GNN layer) with host concat between.
"""
import os
import time
import numpy as np
import ml_dtypes
import jax
from jax.sharding import Mesh, PartitionSpec, NamedSharding
from jax.experimental.shard_map import shard_map
import concourse.bass as bass
import concourse.bacc as bacc
import concourse.mybir as mybir
import concourse.tile as tile
from concourse import library_config, bass2jax
from concourse.bass2jax import _bass_exec_p, install_neuronx_cc_hook
from contextlib import ExitStack

bf16 = ml_dtypes.bfloat16
f32 = np.float32
dt = mybir.dt
A = mybir.ActivationFunctionType
O = mybir.AluOpType
SKIP = set()

N = 50000
D = 128
HEADS = 4
OUT = 32
N_CORES = 8
OWN = N // N_CORES            # 6250
NBLK = (OWN + 127) // 128     # 49
BUCKET = 32768
GCALL = 4096                  # idxs per dma_gather call
CB = 8                        # tiles per DVE batch
SLOPE = 0.2


def plan_core_groups(src, dst, core):
    """Per-(bucket, block) edge lists for one core."""
    base = core * OWN
    sel = (dst >= base) & (dst < base + OWN)
    es = src[sel].astype(np.int64)
    ed = (dst[sel] - base).astype(np.int64)
    groups = {}
    for b in range(2):
        for k in range(NBLK):
            m = ((es >= BUCKET) == bool(b)) & (ed // 128 == k)
            groups[(b, k)] = (es[m] - (BUCKET if b else 0), ed[m] - k * 128)
    return groups


def make_plans(src, dst):
    """Common-structure plans for all cores (same NEFF across cores)."""
    all_groups = [plan_core_groups(src, dst, c) for c in range(N_CORES)]
    TPC = GCALL // 128
    # common tile count per (bucket, block)
    ntiles = {}
    for b in range(2):
        for k in range(NBLK):
            mx = max((len(g[(b, k)][0]) + 127) // 128 for g in all_groups)
            ntiles[(b, k)] = mx
    # pad each bucket-run to a multiple of TPC by growing the last block
    run_lens = [sum(ntiles[(b, k)] for k in range(NBLK)) for b in range(2)]
    for b in range(2):
        pad = (-run_lens[b]) % TPC
        ntiles[(b, NBLK - 1)] += pad
        run_lens[b] += pad
    # common tile records
    blocks, start, end = [], [], []
    for b in range(2):
        for k in range(NBLK):
            n = ntiles[(b, k)]
            if n == 0:
                continue
            blocks += [k] * n
            start += [True] + [False] * (n - 1)
            end += [False] * (n - 1) + [True]
    T = len(blocks)
    n_run0 = run_lens[0]
    common = dict(T=T, n_run0=n_run0, blocks=blocks,
                  start=np.array(start), end=np.array(end))

    plans = []
    for c in range(N_CORES):
        g = all_groups[c]
        idx_src, idx_dst, dstloc_cols = [], [], []
        for b in range(2):
            for k in range(NBLK):
                n = ntiles[(b, k)]
                if n == 0:
                    continue
                e_s, e_d = g[(b, k)]
                cap = n * 128
                pad = cap - len(e_s)
                e_s = np.concatenate([e_s, np.zeros(pad, np.int64)])
                e_dl = np.concatenate([e_d, np.full(pad, 255, np.int64)])
                e_dr = np.concatenate([np.minimum(e_d + k * 128, OWN - 1),
                                       np.zeros(pad, np.int64)])
                idx_src.append(e_s)
                idx_dst.append(e_dr)
                dstloc_cols.append(e_dl.reshape(n, 128))
        p = dict(common)
        p["idx_src"] = np.concatenate(idx_src).astype(np.int16)
        p["idx_dst"] = np.concatenate(idx_dst).astype(np.int16)
        p["dstloc"] = np.concatenate(dstloc_cols, axis=0).T.astype(
            np.float32).astype(bf16)
        p["own_base"] = c * OWN
        plans.append(p)
    return plans


def build_layer(plan, layer):
    """Build the per-core NEFF for one GATv2 layer given the edge plan."""
    T = plan["T"]
    blocks, tstart, tend = plan["blocks"], plan["start"], plan["end"]
    n_run0 = plan["n_run0"]
    NIDX = T * 128
    NCHUNK = NIDX // GCALL
    chunks0 = n_run0 * 128 // GCALL   # chunks in bucket0 run

    nc = bacc.Bacc("TRN2", target_bir_lowering=False, debug=False,
                   num_devices=N_CORES)
    featsT = nc.dram_tensor("featsT", (128, N), dt.bfloat16, kind="ExternalInput")
    feats_own = nc.dram_tensor("feats_own", (OWN, 128), dt.float32, kind="ExternalInput")
    featsT_own = nc.dram_tensor("featsT_own", (128, OWN), dt.bfloat16, kind="ExternalInput")
    W_src = nc.dram_tensor("W_src", (128, 128), dt.bfloat16, kind="ExternalInput")
    W_dst = nc.dram_tensor("W_dst", (128, 128), dt.bfloat16, kind="ExternalInput")
    bc_row = nc.dram_tensor("bc_row", (1, 128), dt.bfloat16, kind="ExternalInput")
    idx_src_d = nc.dram_tensor("idx_src", (128, NIDX // 16), dt.int16, kind="ExternalInput")
    idx_dst_d = nc.dram_tensor("idx_dst", (128, NIDX // 16), dt.int16, kind="ExternalInput")
    dstloc_d = nc.dram_tensor("dstloc", (128, T), dt.bfloat16, kind="ExternalInput")
    attn_rep_d = nc.dram_tensor("attn_rep", (128, 128), dt.bfloat16, kind="ExternalInput")
    iota_rep_d = nc.dram_tensor("iota_rep", (128, 128), dt.bfloat16, kind="ExternalInput")
    bias_rep_d = nc.dram_tensor("bias_rep", (128, 128), dt.float32, kind="ExternalInput")
    bs_rep_d = nc.dram_tensor("bs_rep", (128, 128), dt.float32, kind="ExternalInput")
    ones_row_d = nc.dram_tensor("ones_row", (1, 128), dt.bfloat16, kind="ExternalInput")
    OUTW = 128 if layer == 0 else OUT
    out_d = nc.dram_tensor("out_own", (OWN, OUTW), dt.float32, kind="ExternalOutput")

    with tile.TileContext(nc) as tc, ExitStack() as ctx:
        cpool = ctx.enter_context(tc.tile_pool(name="const", bufs=1))
        ipool = ctx.enter_context(tc.tile_pool(name="idx", bufs=1))
        gpool = ctx.enter_context(tc.tile_pool(name="gath", bufs=2))
        upool = ctx.enter_context(tc.tile_pool(name="u", bufs=3))
        spool = ctx.enter_context(tc.tile_pool(name="scratch", bufs=3))
        apool = ctx.enter_context(tc.tile_pool(name="acc", bufs=1))
        ppool = ctx.enter_context(tc.tile_pool(name="psum", bufs=2, space="PSUM"))
        pjpool = ctx.enter_context(tc.tile_pool(name="psumproj", bufs=2, space="PSUM"))
        drpool = ctx.enter_context(tc.tile_pool(name="dram", bufs=1, space="DRAM"))
        fpool = ctx.enter_context(tc.tile_pool(name="fin", bufs=2))
        f1pool = ctx.enter_context(tc.tile_pool(name="fin1", bufs=1))

        nc.gpsimd.load_library(library_config.mlp)

        # ---------------- constants ----------------
        attn_rep = cpool.tile([128, 128], dt.bfloat16)
        iota_rep = cpool.tile([128, 128], dt.bfloat16)
        bias_rep = cpool.tile([128, 128], dt.float32)
        bs_rep = cpool.tile([128, 128], dt.float32)
        ones_row = cpool.tile([1, 128], dt.bfloat16)
        bc_sb = cpool.tile([1, 128], dt.bfloat16)
        dstloc_sb = cpool.tile([128, T], dt.bfloat16)
        nc.sync.dma_start(attn_rep[:], attn_rep_d[:])
        nc.sync.dma_start(iota_rep[:], iota_rep_d[:])
        nc.sync.dma_start(bias_rep[:], bias_rep_d[:])
        nc.sync.dma_start(bs_rep[:], bs_rep_d[:])
        nc.sync.dma_start(ones_row[:], ones_row_d[:])
        nc.sync.dma_start(bc_sb[:], bc_row[:])
        nc.sync.dma_start(dstloc_sb[:], dstloc_d[:])
        idx_src_sb = ipool.tile([128, NIDX // 16], dt.int16)
        idx_dst_sb = ipool.tile([128, NIDX // 16], dt.int16)
        nc.sync.dma_start(idx_src_sb[:], idx_src_d[:])
        nc.sync.dma_start(idx_dst_sb[:], idx_dst_d[:])

        # ---------------- projection: build tables ----------------
        # fsrc table (all N nodes, unbiased):  row-major bf16 [N, 128]
        tbl_src = drpool.tile([N, 128], dt.bfloat16)
        # fdst table (own nodes, bias bc folded in): bf16 [OWN, 128]
        tbl_dst = drpool.tile([OWN, 128], dt.bfloat16)

        Wsrc_sb = cpool.tile([128, 128], dt.bfloat16)
        Wdst_sb = cpool.tile([128, 128], dt.bfloat16)
        nc.sync.dma_start(Wsrc_sb[:], W_src[:])
        nc.sync.dma_start(Wdst_sb[:], W_dst[:])

        nchunk_src = (N + 127) // 128
        FTC = 16                      # featsT cols per big DMA (x128)
        for k0 in range(0, nchunk_src, FTC):
            k1 = min(k0 + FTC, nchunk_src)
            c0 = k0 * 128
            c1 = min(k1 * 128, N)
            ftb = spool.tile([128, FTC * 128], dt.bfloat16, tag="projft")
            nc.sync.dma_start(ftb[:, :c1 - c0], featsT[:, c0:c1])
            ob = spool.tile([128, FTC, 128], dt.bfloat16, tag="projob")
            for k in range(k0, k1):
                w = min(128, N - k * 128)
                ps = pjpool.tile([128, 128], dt.float32, space="PSUM", tag="projps")
                nc.tensor.matmul(out=ps[:w, :], lhsT=ftb[:, (k - k0) * 128:(k - k0) * 128 + w],
                                 rhs=Wsrc_sb[:], start=True, stop=True)
                nc.scalar.activation(ob[:w, k - k0, :], ps[:w, :], A.Copy)
            nfull = (c1 - c0) // 128
            if nfull:
                nc.sync.dma_start(
                    tbl_src[:][c0:c0 + nfull * 128, :].rearrange(
                        "(c p) d -> p c d", p=128),
                    ob[:, :nfull, :])
            tailr = (c1 - c0) - nfull * 128
            if tailr:
                nc.sync.dma_start(tbl_src[:][c0 + nfull * 128:c1, :],
                                  ob[:tailr, nfull, :])

        # fdst table (own nodes): stage all chunks, single write
        obd = f1pool.tile([128, NBLK, 128], dt.bfloat16, tag="projobd")
        for k in range(NBLK):
            c0 = k * 128
            c1 = min(c0 + 128, OWN)
            w = c1 - c0
            ft = spool.tile([128, 128], dt.bfloat16, tag="projftd")
            nc.sync.dma_start(ft[:, :w], featsT_own[:, c0:c1])
            ps = pjpool.tile([128, 128], dt.float32, space="PSUM", tag="projps")
            nc.tensor.matmul(out=ps[:w, :], lhsT=ones_row[:, :w], rhs=bc_sb[:],
                             start=True, stop=False)
            nc.tensor.matmul(out=ps[:w, :], lhsT=ft[:, :w], rhs=Wdst_sb[:],
                             start=False, stop=True)
            nc.scalar.activation(obd[:w, k, :], ps[:w, :], A.Copy)
        # OWN may not be a multiple of 128; write full blocks then the tail
        full = OWN // 128
        nc.sync.dma_start(
            tbl_dst[:][0:full * 128, :].rearrange("(c p) d -> p c d", p=128),
            obd[:, :full, :])
        tail = OWN - full * 128
        if tail:
            nc.sync.dma_start(tbl_dst[:][full * 128:OWN, :], obd[:tail, full, :])

        # ---------------- edge phase ----------------
        acc = apool.tile([128, NBLK * 132], dt.float32)
        nc.vector.memset(acc[:], 0.0)
        negone = cpool.tile([128, 1], dt.float32)
        nc.vector.memset(negone[:], -1.0)

        TPC = GCALL // 128
        ps_cur = None
        for ch in range(NCHUNK):
            fs = gpool.tile([128, TPC, 128], dt.bfloat16, tag="fs")
            fd = gpool.tile([128, TPC, 128], dt.bfloat16, tag="fd")
            if ch < chunks0:
                src_tab = tbl_src[:][0:BUCKET, :]
            else:
                src_tab = tbl_src[:][BUCKET:N, :]
            if "gather" not in SKIP:
              nc.gpsimd.dma_gather(
                out_ap=fs[:], in_ap=src_tab,
                idxs_ap=idx_src_sb[:, ch * (GCALL // 16):(ch + 1) * (GCALL // 16)],
                num_idxs=GCALL, num_idxs_reg=GCALL, elem_size=128,
                single_packet=False)
            if "gather" not in SKIP:
              nc.gpsimd.dma_gather(
                out_ap=fd[:], in_ap=tbl_dst[:][:, :],
                idxs_ap=idx_dst_sb[:, ch * (GCALL // 16):(ch + 1) * (GCALL // 16)],
                num_idxs=GCALL, num_idxs_reg=GCALL, elem_size=128,
                single_packet=False)

            for sb in range(TPC // CB):
                t0 = ch * TPC + sb * CB      # global tile index of batch start
                sl = slice(sb * CB, (sb + 1) * CB)
                u = upool.tile([128, CB, 132], dt.bfloat16, tag="u")
                if "dve" not in SKIP:
                    nc.vector.tensor_tensor(out=u[:, :, 0:128], in0=fs[:, sl, :],
                                            in1=fd[:, sl, :], op=O.add)
                lr = spool.tile([128, CB, 128], dt.bfloat16, tag="lr")
                if "act" not in SKIP:
                    nc.scalar.activation(lr[:], u[:, :, 0:128], A.Prelu, alpha=SLOPE)
                tt = spool.tile([128, CB, 128], dt.float32, tag="tt")
                G = spool.tile([128, CB, 128], dt.bfloat16, tag="G")
                if "dve" not in SKIP:
                    ar3 = attn_rep[:].rearrange("p (o f) -> p o f", o=1)
                    nc.vector.tensor_tensor(out=tt[:], in0=lr[:],
                                            in1=ar3.broadcast_to((128, CB, 128)), op=O.mult)
                    lg = spool.tile([128, CB, 4], dt.float32, tag="lg")
                    nc.vector.tensor_reduce(
                        out=lg[:], in_=tt[:].rearrange("p c (h d) -> p c h d", h=HEADS),
                        axis=mybir.AxisListType.X, op=O.add)
                    if "act" not in SKIP:
                        nc.scalar.activation(u[:, :, 128:132], lg[:], A.Exp)
                    w4 = u[:, :, 128:132].rearrange("p c (h x) -> p c h x", x=1)
                    uh = u[:, :, 0:128].rearrange("p c (h d) -> p c h d", h=HEADS)
                    nc.vector.tensor_tensor(out=uh, in0=uh,
                                            in1=w4.broadcast_to((128, CB, HEADS, 32)),
                                            op=O.mult)
                    io3 = iota_rep[:].rearrange("p (o f) -> p o f", o=1)
                    dl3 = dstloc_sb[:, t0:t0 + CB].rearrange("p (c o) -> p c o", o=1)
                    nc.vector.tensor_tensor(out=G[:], in0=io3.broadcast_to((128, CB, 128)),
                                            in1=dl3.broadcast_to((128, CB, 128)),
                                            op=O.is_equal)
                for c in range(CB):
                    if "mm" in SKIP:
                        break
                    ti = t0 + c
                    if tstart[ti]:
                        ps_cur = ppool.tile([128, 132], dt.float32, space="PSUM",
                                            tag="aggps")
                    nc.tensor.matmul(out=ps_cur[:], lhsT=G[:, c, :], rhs=u[:, c, :],
                                     start=bool(tstart[ti]), stop=bool(tend[ti]))
                    if tend[ti]:
                        k = blocks[ti]
                        nc.vector.tensor_tensor(
                            out=acc[:, k * 132:(k + 1) * 132],
                            in0=acc[:, k * 132:(k + 1) * 132],
                            in1=ps_cur[:], op=O.add)

        # ---------------- finalize ----------------
        accv = acc[:].rearrange("p (b f) -> p b f", f=132)
        den = accv[:, :, 128:132]                      # [128, NBLK, 4]
        rd = fpool.tile([128, NBLK, 4], dt.float32, tag="rd")
        nc.vector.tensor_scalar(out=rd[:], in0=den, scalar1=1e-30, scalar2=None,
                                op0=O.max)
        nc.vector.reciprocal(out=rd[:], in_=rd[:])
        sa = fpool.tile([128, NBLK, 4], dt.float32, tag="sa")
        nc.vector.tensor_tensor(out=sa[:], in0=den, in1=rd[:], op=O.mult)

        full = OWN // 128
        fdall = f1pool.tile([128, NBLK, 128], dt.bfloat16, tag="fdall")
        nc.sync.dma_start(fdall[:, :full, :],
                          tbl_dst[:][0:full * 128, :].rearrange("(c p) d -> p c d", p=128))
        if OWN - full * 128:
            nc.sync.dma_start(fdall[:OWN - full * 128, full, :],
                              tbl_dst[:][full * 128:OWN, :])
        ostage = f1pool.tile([128, NBLK, OUTW], dt.float32, tag="ostage")
        for k in range(NBLK):
            r0 = k * 128
            r1 = min(r0 + 128, OWN)
            w = r1 - r0
            # s = acc[:, :128] * rd  (per-head broadcast)
            s = fpool.tile([128, 128], dt.float32, tag="s")
            rd4 = rd[:, k:k+1, :].rearrange("p o h -> p (o h)")   # [128, 4]
            rdb = rd4.rearrange("p (h x) -> p h x", x=1)
            nc.vector.tensor_tensor(
                out=s[:].rearrange("p (h d) -> p h d", h=HEADS),
                in0=accv[:, k, 0:128].rearrange("p (h d) -> p h d", h=HEADS),
                in1=rdb.broadcast_to((128, HEADS, 32)), op=O.mult)
            # fdst block rows -> fp32
            fdf = fpool.tile([128, 128], dt.float32, tag="fdf")
            nc.scalar.activation(fdf[:w, :], fdall[:w, k, :], A.Copy)
            nc.vector.tensor_tensor(out=fdf[:], in0=fdf[:], in1=bs_rep[:],
                                    op=O.subtract)
            sab = sa[:, k:k+1, :].rearrange("p o h -> p (o h)").rearrange(
                "p (h x) -> p h x", x=1)
            nc.vector.tensor_tensor(
                out=fdf[:].rearrange("p (h d) -> p h d", h=HEADS),
                in0=fdf[:].rearrange("p (h d) -> p h d", h=HEADS),
                in1=sab.broadcast_to((128, HEADS, 32)), op=O.mult)
            # t2 = s - fdst + feats_own + bias_rep
            nc.vector.tensor_tensor(out=s[:w], in0=s[:w], in1=fdf[:w], op=O.subtract)
            fo = fpool.tile([128, 128], dt.float32, tag="fo")
            nc.sync.dma_start(fo[:w, :], feats_own[r0:r1, :])
            nc.vector.tensor_tensor(out=s[:w], in0=s[:w], in1=fo[:w], op=O.add)
            nc.vector.tensor_tensor(out=s[:w], in0=s[:w], in1=bias_rep[:w], op=O.add)
            if layer == 0:
                # y = elu(elu(s)) ; elu(x) = relu(x) + exp(min(x,0)) - 1
                m = fpool.tile([128, 128], dt.float32, tag="m")
                nc.vector.tensor_scalar_min(out=m[:w], in0=s[:w], scalar1=0.0)
                em = fpool.tile([128, 128], dt.float32, tag="em")
                nc.scalar.activation(em[:w], m[:w], A.Exp)
                y1 = fpool.tile([128, 128], dt.float32, tag="y1")
                # y1 = relu(s) + em   (elu1 = y1 - 1)
                nc.vector.scalar_tensor_tensor(out=y1[:w], in0=s[:w], scalar=0.0,
                                               in1=em[:w], op0=O.max, op1=O.add)
                # second elu on (y1-1)
                nc.vector.tensor_scalar_min(out=m[:w], in0=y1[:w], scalar1=1.0)
                nc.scalar.activation(em[:w], m[:w], A.Exp, bias=negone[:w])
                nc.vector.tensor_scalar_max(out=y1[:w], in0=y1[:w], scalar1=1.0)
                nc.vector.scalar_tensor_tensor(out=ostage[:w, k, :], in0=y1[:w],
                                               scalar=-2.0, in1=em[:w],
                                               op0=O.add, op1=O.add)
            else:
                # mean over heads -> [w, 32]
                h01 = fpool.tile([128, 32], dt.float32, tag="h01")
                nc.vector.tensor_tensor(out=h01[:w], in0=s[:w, 0:32],
                                        in1=s[:w, 32:64], op=O.add)
                h23 = fpool.tile([128, 32], dt.float32, tag="h23")
                nc.vector.tensor_tensor(out=h23[:w], in0=s[:w, 64:96],
                                        in1=s[:w, 96:128], op=O.add)
                nc.vector.tensor_tensor(out=h01[:w], in0=h01[:w], in1=h23[:w], op=O.add)
                nc.vector.tensor_scalar_mul(out=ostage[:w, k, :], in0=h01[:w],
                                            scalar1=0.25)

        nc.sync.dma_start(out_d[0:full * 128, :].rearrange("(c p) d -> p c d", p=128),
                          ostage[:, :full, :])
        if OWN - full * 128:
            nc.sync.dma_start(out_d[full * 128:OWN, :],
                              ostage[:OWN - full * 128, full, :])

    nc.compile()
    return nc


# ---------------------------------------------------------------- runner ----
import time
import numpy as np
import jax
from jax.sharding import Mesh, PartitionSpec, NamedSharding
from jax.experimental.shard_map import shard_map
import concourse.mybir as mybir
from concourse import bass2jax
from concourse.bass2jax import _bass_exec_p, install_neuronx_cc_hook

def make_runner(nc, n_cores, use_donate=False):
    install_neuronx_cc_hook()
    partition_name = nc.partition_id_tensor.name if nc.partition_id_tensor else None
    in_names, out_names, out_avals, zero_outs = [], [], [], []
    for alloc in nc.m.functions[0].allocations:
        if not isinstance(alloc, mybir.MemoryLocationSet):
            continue
        name = alloc.memorylocations[0].name
        if alloc.kind == "ExternalInput":
            if name != partition_name:
                in_names.append(name)
        elif alloc.kind == "ExternalOutput":
            dt = mybir.dt.np(alloc.dtype)
            out_avals.append(jax.core.ShapedArray(tuple(alloc.tensor_shape), dt))
            out_names.append(name)
            zero_outs.append(np.zeros(tuple(alloc.tensor_shape), dt))
    n_params = len(in_names)
    n_outs = len(out_names)
    in_names.extend(out_names)
    if partition_name is not None:
        in_names.append(partition_name)
    donate = tuple(range(n_params, n_params + n_outs))

    def _body(*args):
        operands = list(args)
        if partition_name is not None:
            operands.append(bass2jax.partition_id_tensor())
        outs = _bass_exec_p.bind(
            *operands, out_avals=tuple(out_avals), in_names=tuple(in_names),
            out_names=tuple(out_names), lowering_input_output_aliases=(),
            sim_require_finite=True, sim_require_nnan=True, nc=nc)
        return tuple(outs)

    devices = jax.devices()[:n_cores]
    mesh = Mesh(np.asarray(devices), ("core",))
    sharded = jax.jit(
        shard_map(_body, mesh=mesh,
                  in_specs=(PartitionSpec("core"),) * (n_params + n_outs),
                  out_specs=(PartitionSpec("core"),) * n_outs,
                  check_rep=False),
        donate_argnums=(donate if use_donate else ()), keep_unused=True)

    class Runner:
        def __init__(self):
            self.in_names = in_names; self.out_names = out_names
            self.real_in_names = in_names[:n_params]
            self.out_avals = out_avals; self.n_cores = n_cores
        def prep(self, in_maps):
            concat = [np.concatenate([m[nm] for m in in_maps], axis=0) for nm in self.real_in_names]
            concat += [np.concatenate([z]*n_cores, axis=0) for z in zero_outs]
            sh = NamedSharding(mesh, PartitionSpec("core"))
            return [jax.device_put(a, sh) for a in concat]
        def run(self, dev_args):
            return sharded(*dev_args)
        def run_np(self, in_maps):
            outs = self.run(self.prep(in_maps))
            return [
                {nm: np.asarray(outs[i]).reshape(n_cores, *out_avals[i].shape)[c]
                 for i, nm in enumerate(out_names)}
                for c in range(n_cores)]
        def time_steady(self, dev_args, iters=6, warmup=2):
            for _ in range(warmup):
                jax.block_until_ready(self.run(dev_args))
            ts = []
            for _ in range(iters):
                t0 = time.perf_counter()
                jax.block_until_ready(self.run(dev_args))
                ts.append(time.perf_counter() - t0)
            return min(ts), ts
    return Runner()


# ------------------------------------------------------------- host glue ----
def make_consts(attn, bias_row, bc_row_vals):
    """Per-layer constant tensors. attn [H, OUT] fp32; bias_row [128] fp32
    (out-bias + b_src); bc_row_vals [128] fp32 (b_src + b_dst)."""
    attn_rep = np.tile(attn.reshape(1, -1), (128, 1)).astype(bf16)
    iota_rep = np.tile(np.arange(128, dtype=f32)[None, :], (128, 1)).astype(bf16)
    bias_rep = np.tile(bias_row.reshape(1, -1), (128, 1)).astype(f32)
    ones_row = np.ones((1, 128), bf16)
    bc_row = bc_row_vals.reshape(1, -1).astype(bf16)
    return attn_rep, iota_rep, bias_rep, ones_row, bc_row


def layer_in_maps(plans, featsT_bf, feats_full, W_src, W_dst, b_src, b_dst,
                  attn, bias):
    """Build per-core in_maps for one layer launch."""
    bc_vals = (b_src + b_dst).astype(f32)
    bias_row = bias.astype(f32)
    attn_rep, iota_rep, bias_rep, ones_row, bc_row = make_consts(
        attn, bias_row, bc_vals)
    Ws = W_src.astype(bf16)
    Wd = W_dst.astype(bf16)
    in_maps = []
    for c, p in enumerate(plans):
        base = c * OWN
        def wrap16rep(a):
            return np.tile(a.reshape(-1, 16).T, (8, 1)).copy()
        in_maps.append(dict(
            featsT=featsT_bf,
            featsT_own=np.ascontiguousarray(featsT_bf[:, base:base + OWN]),
            feats_own=feats_full[base:base + OWN].astype(f32),
            W_src=Ws, W_dst=Wd, bc_row=bc_row,
            idx_src=wrap16rep(p["idx_src"]), idx_dst=wrap16rep(p["idx_dst"]), dstloc=p["dstloc"],
            attn_rep=attn_rep, iota_rep=iota_rep, bias_rep=bias_rep,
            bs_rep=np.tile(b_src.reshape(1, -1), (128, 1)).astype(f32),
            ones_row=ones_row,
        ))
    return in_maps


class TwoLayerRunner:
    def __init__(self, src, dst, verbose=False):
        self.plans = make_plans(src, dst)
        self.T = self.plans[0]["T"]
        if verbose:
            print(f"common T={self.T} tiles ({self.T*128} idx slots)")
        self.nc0 = build_layer(self.plans[0], layer=0)
        self.nc1 = build_layer(self.plans[0], layer=1)
        self.r0 = make_runner(self.nc0, N_CORES)
        self.r1 = make_runner(self.nc1, N_CORES)

    def __call__(self, feats, inp):
        featsT_bf = np.ascontiguousarray(feats.T).astype(bf16)
        m0 = layer_in_maps(self.plans, featsT_bf, feats,
                           inp["W_src0"], inp["W_dst0"], inp["b_src0"][:],
                           inp["b_dst0"][:], inp["attn0"], inp["bias0"])
        outs0 = self.r0.run_np(m0)
        h1 = np.concatenate([o["out_own"] for o in outs0], axis=0)  # [N, 128]
        h1T_bf = np.ascontiguousarray(h1.T).astype(bf16)
        m1 = layer_in_maps(self.plans, h1T_bf, h1,
                           inp["W_src1"], inp["W_dst1"], inp["b_src1"][:],
                           inp["b_dst1"][:], inp["attn1"], inp["bias1"])
        outs1 = self.r1.run_np(m1)
        out = np.concatenate([o["out_own"] for o in outs1], axis=0)  # [N, 32]
        return h1, out


_TLR_CACHE = {}


def kernel(**inputs):
    inputs = {k: np.asarray(v) for k, v in inputs.items()}
    src = inputs["src"].astype(np.int64)
    dst = inputs["dst"].astype(np.int64)
    feats = inputs["feats"].astype(np.float32)
    kh = hash((src.tobytes(), dst.tobytes()))
    if kh not in _TLR_CACHE:
        _TLR_CACHE[kh] = TwoLayerRunner(src, dst)
    tlr = _TLR_CACHE[kh]
    _h1, out = tlr(feats, inputs)
    return out.astype(np.float32)


_NULL_CACHE = {}


def null_baseline():
    """Steady-state wall of a near-empty 8-core launch (dispatch overhead)."""
    if "t" in _NULL_CACHE:
        return _NULL_CACHE["t"]
    nc = bacc.Bacc("TRN2", target_bir_lowering=False, debug=False,
                   num_devices=N_CORES)
    x = nc.dram_tensor("x", (128, 128), dt.float32, kind="ExternalInput")
    y = nc.dram_tensor("y", (128, 128), dt.float32, kind="ExternalOutput")
    with tile.TileContext(nc) as tc, ExitStack() as ctx:
        pool = ctx.enter_context(tc.tile_pool(name="sbuf", bufs=2))
        t = pool.tile([128, 128], dt.float32)
        nc.sync.dma_start(t[:], x[:])
        nc.sync.dma_start(y[:], t[:])
    nc.compile()
    r = make_runner(nc, N_CORES)
    xs = np.zeros((128, 128), np.float32)
    dev = r.prep([{"x": xs}] * N_CORES)
    best, _ = r.time_steady(dev, iters=8, warmup=2)
    _NULL_CACHE["t"] = best
    return best



# revision 7
# speedup vs baseline: 1.2085x; 1.2085x over previous
"""Trainium2 Bass kernel for nn_GATv2_23278722744604.

2-layer GATv2 (N=50000 nodes, E=800000 edges, 128 feats, 4 heads x 32).
Sharding: destination-node blocks across 8 NeuronCores; edges routed to the
owner of their dst node. Per-edge source features fetched by dma_gather from a
replicated projected table (bf16). Dst features are broadcast to edges on the
PE via one-hot matmuls (no dst gather): per tile, GT = transpose(G) on PE,
uT = obd_k @ GT + fs^T accumulated in PSUM, LeakyReLU on the scalar engine,
per-head logits via a 4-column matmul, softmax numerator aggregated with a
one-hot scatter matmul into PSUM. Two launches (one per GNN layer) with host
concat between.
"""
import os
import time
import numpy as np
import ml_dtypes
import jax
from jax.sharding import Mesh, PartitionSpec, NamedSharding
from jax.experimental.shard_map import shard_map
import concourse.bass as bass
import concourse.bacc as bacc
import concourse.mybir as mybir
import concourse.tile as tile
from concourse import library_config, bass2jax
from concourse.bass2jax import _bass_exec_p, install_neuronx_cc_hook
from contextlib import ExitStack

bf16 = ml_dtypes.bfloat16
f32 = np.float32
dt = mybir.dt
A = mybir.ActivationFunctionType
O = mybir.AluOpType
SKIP = set()

N = 50000
D = 128
HEADS = 4
OUT = 32
N_CORES = 8
OWN = N // N_CORES            # 6250
NBLK = (OWN + 127) // 128     # 49
BUCKET = 32768
GCALL = 4096                  # idxs per dma_gather call
CB = 8                        # tiles per DVE batch
SLOPE = 0.2


def plan_core_groups(src, dst, core):
    """Per-(bucket, block) edge lists for one core."""
    base = core * OWN
    sel = (dst >= base) & (dst < base + OWN)
    es = src[sel].astype(np.int64)
    ed = (dst[sel] - base).astype(np.int64)
    groups = {}
    for b in range(2):
        for k in range(NBLK):
            m = ((es >= BUCKET) == bool(b)) & (ed // 128 == k)
            groups[(b, k)] = (es[m] - (BUCKET if b else 0), ed[m] - k * 128)
    return groups


def make_plans(src, dst):
    """Common-structure plans for all cores (same NEFF across cores)."""
    all_groups = [plan_core_groups(src, dst, c) for c in range(N_CORES)]
    TPC = GCALL // 128
    # common tile count per (bucket, block)
    ntiles = {}
    for b in range(2):
        for k in range(NBLK):
            mx = max((len(g[(b, k)][0]) + 127) // 128 for g in all_groups)
            ntiles[(b, k)] = mx
    # pad each bucket-run to a multiple of TPC by growing the last block
    run_lens = [sum(ntiles[(b, k)] for k in range(NBLK)) for b in range(2)]
    for b in range(2):
        pad = (-run_lens[b]) % TPC
        ntiles[(b, NBLK - 1)] += pad
        run_lens[b] += pad
    # common tile records
    blocks, start, end = [], [], []
    for b in range(2):
        for k in range(NBLK):
            n = ntiles[(b, k)]
            if n == 0:
                continue
            blocks += [k] * n
            start += [True] + [False] * (n - 1)
            end += [False] * (n - 1) + [True]
    T = len(blocks)
    n_run0 = run_lens[0]
    common = dict(T=T, n_run0=n_run0, blocks=blocks,
                  start=np.array(start), end=np.array(end))

    plans = []
    for c in range(N_CORES):
        g = all_groups[c]
        idx_src, dstloc_cols = [], []
        for b in range(2):
            for k in range(NBLK):
                n = ntiles[(b, k)]
                if n == 0:
                    continue
                e_s, e_d = g[(b, k)]
                cap = n * 128
                pad = cap - len(e_s)
                e_s = np.concatenate([e_s, np.zeros(pad, np.int64)])
                e_dl = np.concatenate([e_d, np.full(pad, 255, np.int64)])
                idx_src.append(e_s)
                dstloc_cols.append(e_dl.reshape(n, 128))
        p = dict(common)
        p["idx_src"] = np.concatenate(idx_src).astype(np.int16)
        p["dstloc"] = np.concatenate(dstloc_cols, axis=0).T.astype(
            np.float32).astype(bf16)
        p["own_base"] = c * OWN
        plans.append(p)
    return plans


def build_layer(plan, layer):
    """Build the per-core NEFF for one GATv2 layer given the edge plan."""
    T = plan["T"]
    blocks, tstart, tend = plan["blocks"], plan["start"], plan["end"]
    n_run0 = plan["n_run0"]
    NIDX = T * 128
    NCHUNK = NIDX // GCALL
    chunks0 = n_run0 * 128 // GCALL   # chunks in bucket0 run
    TPC = GCALL // 128

    nc = bacc.Bacc("TRN2", target_bir_lowering=False, debug=False,
                   num_devices=N_CORES)
    featsT = nc.dram_tensor("featsT", (128, N), dt.bfloat16, kind="ExternalInput")
    feats_own = nc.dram_tensor("feats_own", (OWN, 128), dt.float32, kind="ExternalInput")
    featsT_own = nc.dram_tensor("featsT_own", (128, OWN), dt.bfloat16, kind="ExternalInput")
    W_src = nc.dram_tensor("W_src", (128, 128), dt.bfloat16, kind="ExternalInput")
    W_dst = nc.dram_tensor("W_dst", (128, 128), dt.bfloat16, kind="ExternalInput")
    bc_row = nc.dram_tensor("bc_row", (1, 128), dt.bfloat16, kind="ExternalInput")
    idx_src_d = nc.dram_tensor("idx_src", (128, NIDX // 16), dt.int16, kind="ExternalInput")
    dstloc_d = nc.dram_tensor("dstloc", (128, T), dt.bfloat16, kind="ExternalInput")
    attnT_d = nc.dram_tensor("attnT", (128, HEADS), dt.bfloat16, kind="ExternalInput")
    iota_rep_d = nc.dram_tensor("iota_rep", (128, 128), dt.bfloat16, kind="ExternalInput")
    ident_d = nc.dram_tensor("ident", (128, 128), dt.bfloat16, kind="ExternalInput")
    ones_row_d = nc.dram_tensor("ones_row", (1, 128), dt.bfloat16, kind="ExternalInput")
    OUTW = 128 if layer == 0 else OUT
    out_d = nc.dram_tensor("out_own", (OWN, OUTW), dt.float32, kind="ExternalOutput")

    with tile.TileContext(nc) as tc, ExitStack() as ctx:
        cpool = ctx.enter_context(tc.tile_pool(name="const", bufs=1))
        ipool = ctx.enter_context(tc.tile_pool(name="idx", bufs=1))
        gpool = ctx.enter_context(tc.tile_pool(name="gath", bufs=2))
        upool = ctx.enter_context(tc.tile_pool(name="u", bufs=3))
        spool = ctx.enter_context(tc.tile_pool(name="scratch", bufs=3))
        pjspool = ctx.enter_context(tc.tile_pool(name="projsb", bufs=2))
        apool = ctx.enter_context(tc.tile_pool(name="acc", bufs=1))
        ppool = ctx.enter_context(tc.tile_pool(name="psum", bufs=2, space="PSUM"))
        pspool = ctx.enter_context(tc.tile_pool(name="pst", bufs=2, space="PSUM"))
        plpool = ctx.enter_context(tc.tile_pool(name="plg", bufs=1, space="PSUM"))
        pjpool = ctx.enter_context(tc.tile_pool(name="psumproj", bufs=1, space="PSUM"))
        drpool = ctx.enter_context(tc.tile_pool(name="dram", bufs=1, space="DRAM"))
        fpool = ctx.enter_context(tc.tile_pool(name="fin", bufs=1))
        f1pool = ctx.enter_context(tc.tile_pool(name="fin1", bufs=1))

        nc.gpsimd.load_library(library_config.mlp)

        # ---------------- constants ----------------
        attnT = cpool.tile([128, HEADS], dt.bfloat16)
        iota_rep = cpool.tile([128, 128], dt.bfloat16)
        ident = cpool.tile([128, 128], dt.bfloat16)
        ones_row = cpool.tile([1, 128], dt.bfloat16)
        bc_sb = cpool.tile([1, 128], dt.bfloat16)
        dstloc_sb = cpool.tile([128, T], dt.bfloat16)
        nc.sync.dma_start(attnT[:], attnT_d[:])
        nc.sync.dma_start(iota_rep[:], iota_rep_d[:])
        nc.sync.dma_start(ident[:], ident_d[:])
        nc.sync.dma_start(ones_row[:], ones_row_d[:])
        nc.sync.dma_start(bc_sb[:], bc_row[:])
        nc.sync.dma_start(dstloc_sb[:], dstloc_d[:])
        idx_src_sb = ipool.tile([128, NIDX // 16], dt.int16)
        nc.sync.dma_start(idx_src_sb[:], idx_src_d[:])

        # ---------------- projection: build tables ----------------
        # fsrc table (all N nodes, unbiased):  row-major bf16 [N, 128]
        tbl_src = drpool.tile([N, 128], dt.bfloat16)

        Wsrc_sb = cpool.tile([128, 128], dt.bfloat16)
        Wdst_sb = cpool.tile([128, 128], dt.bfloat16)
        nc.sync.dma_start(Wsrc_sb[:], W_src[:])
        nc.sync.dma_start(Wdst_sb[:], W_dst[:])

        nchunk_src = (N + 127) // 128
        FTC = 16                      # featsT cols per big DMA (x128)
        PJB = 4                       # psum tiles per scalar copy batch
        for k0 in range(0, nchunk_src, FTC):
            k1 = min(k0 + FTC, nchunk_src)
            c0 = k0 * 128
            c1 = min(k1 * 128, N)
            ftb = pjspool.tile([128, FTC * 128], dt.bfloat16, tag="projft")
            nc.sync.dma_start(ftb[:, :c1 - c0], featsT[:, c0:c1])
            ob = pjspool.tile([128, FTC, 128], dt.bfloat16, tag="projob")
            for g0 in range(k0, k1, PJB):
                g1 = min(g0 + PJB, k1)
                ps = pjpool.tile([128, PJB, 128], dt.float32, space="PSUM",
                                 tag="projps")
                for k in range(g0, g1):
                    w = min(128, N - k * 128)
                    nc.tensor.matmul(out=ps[:w, k - g0, :],
                                     lhsT=ftb[:, (k - k0) * 128:(k - k0) * 128 + w],
                                     rhs=Wsrc_sb[:], start=True, stop=True)
                wmax = min(128, N - g0 * 128)
                nc.scalar.activation(ob[:wmax, g0 - k0:g1 - k0, :],
                                     ps[:wmax, :g1 - g0, :], A.Copy)
            nfull = (c1 - c0) // 128
            if nfull:
                nc.sync.dma_start(
                    tbl_src[:][c0:c0 + nfull * 128, :].rearrange(
                        "(c p) d -> p c d", p=128),
                    ob[:, :nfull, :])
            tailr = (c1 - c0) - nfull * 128
            if tailr:
                nc.sync.dma_start(tbl_src[:][c0 + nfull * 128:c1, :],
                                  ob[:tailr, nfull, :])

        # fdst table (own nodes, bias bc folded in): bf16 [128, NBLK, 128]
        # kept resident in SBUF; obd[:, k, :] = rows of dst block k.
        obd = f1pool.tile([128, NBLK, 128], dt.bfloat16, tag="projobd")
        if OWN % 128:
            # zero the last block before valid rows are written over it: pad
            # rows feed the fdT matmul (against all-zero one-hot columns, but
            # NaN*0 would poison it)
            nc.vector.memset(obd[:, NBLK - 1, :], 0.0)
        for k in range(NBLK):
            c0 = k * 128
            c1 = min(c0 + 128, OWN)
            w = c1 - c0
            ft = spool.tile([128, 128], dt.bfloat16, tag="projftd")
            nc.sync.dma_start(ft[:, :w], featsT_own[:, c0:c1])
            ps = pjpool.tile([128, PJB, 128], dt.float32, space="PSUM", tag="projps")
            nc.tensor.matmul(out=ps[:w, 0, :], lhsT=ones_row[:, :w], rhs=bc_sb[:],
                             start=True, stop=False)
            nc.tensor.matmul(out=ps[:w, 0, :], lhsT=ft[:, :w], rhs=Wdst_sb[:],
                             start=False, stop=True)
            nc.scalar.activation(obd[:w, k, :], ps[:w, 0, :], A.Copy)
        if OWN % 128:
            # pad rows (unused dst slots) don't matter; leave uninitialized-free
            pass

        # ---------------- edge phase ----------------
        acc = apool.tile([128, NBLK * 132], dt.float32)
        nc.vector.memset(acc[:], 0.0)
        negone = cpool.tile([128, 1], dt.float32)
        nc.vector.memset(negone[:], -1.0)

        ps_cur = None
        for ch in range(NCHUNK):
            fs = gpool.tile([128, TPC, 128], dt.bfloat16, tag="fs")
            if ch < chunks0:
                src_tab = tbl_src[:][0:BUCKET, :]
            else:
                src_tab = tbl_src[:][BUCKET:N, :]
            if "gather" not in SKIP:
                nc.gpsimd.dma_gather(
                    out_ap=fs[:], in_ap=src_tab,
                    idxs_ap=idx_src_sb[:, ch * (GCALL // 16):(ch + 1) * (GCALL // 16)],
                    num_idxs=GCALL, num_idxs_reg=GCALL, elem_size=128,
                    single_packet=False)

            for sb in range(TPC // CB):
                t0 = ch * TPC + sb * CB      # global tile index of batch start
                sl = slice(sb * CB, (sb + 1) * CB)
                # one-hot G[e, j] per tile (DVE)
                G = spool.tile([128, CB, 128], dt.bfloat16, tag="G")
                if "dve" not in SKIP:
                    io3 = iota_rep[:].rearrange("p (o f) -> p o f", o=1)
                    dl3 = dstloc_sb[:, t0:t0 + CB].rearrange("p (c o) -> p c o", o=1)
                    nc.vector.tensor_tensor(out=G[:], in0=io3.broadcast_to((128, CB, 128)),
                                            in1=dl3.broadcast_to((128, CB, 128)),
                                            op=O.is_equal)
                # GT = transpose(G) on PE, copy to SBUF (scalar)
                gt_ps = pspool.tile([128, CB, 128], dt.float32, space="PSUM",
                                    tag="pst")
                if "mm" not in SKIP:
                    for c in range(CB):
                        nc.tensor.matmul(out=gt_ps[:, c, :], lhsT=G[:, c, :],
                                         rhs=ident[:], start=True, stop=True)
                GTsb = spool.tile([128, CB, 128], dt.bfloat16, tag="GTsb")
                if "act" not in SKIP:
                    nc.scalar.activation(GTsb[:], gt_ps[:], A.Copy)
                # uT = obd_k @ GT + fs^T  (PSUM accumulate, feature-major)
                ut_ps = pspool.tile([128, CB, 128], dt.float32, space="PSUM",
                                    tag="pst")
                if "mm" not in SKIP:
                    for c in range(CB):
                        k = blocks[t0 + c]
                        nc.tensor.matmul(out=ut_ps[:, c, :], lhsT=obd[:, k, :],
                                         rhs=GTsb[:, c, :], start=True, stop=False)
                        nc.tensor.matmul(out=ut_ps[:, c, :], lhsT=fs[:, sb * CB + c, :],
                                         rhs=ident[:], start=False, stop=True)
                # lrT = LeakyReLU(uT) (scalar), then per-head logits on PE
                lrT = spool.tile([128, CB, 128], dt.bfloat16, tag="lrT")
                if "act" not in SKIP:
                    nc.scalar.activation(lrT[:], ut_ps[:], A.Prelu, alpha=SLOPE)
                lg_ps = plpool.tile([128, CB, HEADS], dt.float32, space="PSUM",
                                    tag="plg")
                if "mm" not in SKIP:
                    for c in range(CB):
                        nc.tensor.matmul(out=lg_ps[:, c, :], lhsT=lrT[:, c, :],
                                         rhs=attnT[:], start=True, stop=True)
                # w = [fs * ex | ex]
                w = upool.tile([128, CB, 132], dt.bfloat16, tag="w")
                if "act" not in SKIP:
                    nc.scalar.activation(w[:, :, 128:132], lg_ps[:], A.Exp)
                if "dve" not in SKIP:
                    w4 = w[:, :, 128:132].rearrange("p c (h x) -> p c h x", x=1)
                    wh = w[:, :, 0:128].rearrange("p c (h d) -> p c h d", h=HEADS)
                    fsh = fs[:, sl, :].rearrange("p c (h d) -> p c h d", h=HEADS)
                    nc.vector.tensor_tensor(out=wh, in0=fsh,
                                            in1=w4.broadcast_to((128, CB, HEADS, 32)),
                                            op=O.mult)
                # scatter-aggregate into per-block PSUM
                for c in range(CB):
                    if "mm" in SKIP:
                        break
                    ti = t0 + c
                    if tstart[ti]:
                        ps_cur = ppool.tile([128, 132], dt.float32, space="PSUM",
                                            tag="aggps")
                    nc.tensor.matmul(out=ps_cur[:], lhsT=G[:, c, :], rhs=w[:, c, :],
                                     start=bool(tstart[ti]), stop=bool(tend[ti]))
                    if tend[ti]:
                        k = blocks[ti]
                        nc.vector.tensor_tensor(
                            out=acc[:, k * 132:(k + 1) * 132],
                            in0=acc[:, k * 132:(k + 1) * 132],
                            in1=ps_cur[:], op=O.add)

        # ---------------- finalize (chunked batches of blocks) ----------------
        accv = acc[:].rearrange("p (b f) -> p b f", f=132)
        den = accv[:, :, 128:132]                      # [128, NBLK, 4]
        rd = f1pool.tile([128, NBLK, 4], dt.float32, tag="rd")
        nc.vector.tensor_scalar(out=rd[:], in0=den, scalar1=1e-30, scalar2=None,
                                op0=O.max)
        nc.vector.reciprocal(out=rd[:], in_=rd[:])
        rdb = rd[:].rearrange("p b (h x) -> p b h x", x=1)
        full = OWN // 128

        FB = 13                                        # blocks per finalize chunk
        for b0 in range(0, NBLK, FB):
            b1 = min(b0 + FB, NBLK)
            nb = b1 - b0
            # fo = feats_own + bias' (host-folded)
            fo = fpool.tile([128, FB, 128], dt.float32, tag="fo")
            r0 = b0 * 128
            r1 = min(b1 * 128, OWN)
            nfull = max(0, min(b1, full) - b0)
            if nfull:
                nc.sync.dma_start(
                    fo[:, :nfull, :],
                    feats_own[r0:r0 + nfull * 128, :].rearrange(
                        "(c p) d -> p c d", p=128))
            tailr = (r1 - r0) - nfull * 128
            if tailr > 0:
                nc.sync.dma_start(fo[:tailr, nfull, :],
                                  feats_own[r0 + nfull * 128:r1, :])
            # s = acc * rd (per-head) + fo
            s = fpool.tile([128, FB, 128], dt.float32, tag="s")
            nc.vector.tensor_tensor(
                out=s[:, :nb].rearrange("p b (h d) -> p b h d", h=HEADS),
                in0=accv[:, b0:b1, 0:128].rearrange("p b (h d) -> p b h d", h=HEADS),
                in1=rdb[:, b0:b1].broadcast_to((128, nb, HEADS, 32)), op=O.mult)
            nc.vector.tensor_tensor(out=s[:, :nb], in0=s[:, :nb], in1=fo[:, :nb],
                                    op=O.add)

            ost = fpool.tile([128, FB, OUTW], dt.float32, tag="ostage")
            if layer == 0:
                # y = elu(elu(s)) ; elu(x) = relu(x) + exp(min(x,0)) - 1
                m = fpool.tile([128, FB, 128], dt.float32, tag="m")
                nc.vector.tensor_scalar_min(out=m[:, :nb], in0=s[:, :nb], scalar1=0.0)
                em = fpool.tile([128, FB, 128], dt.float32, tag="em")
                nc.scalar.activation(em[:, :nb], m[:, :nb], A.Exp)
                # y1 = relu(s) + em   (elu1 = y1 - 1); write into s
                nc.vector.scalar_tensor_tensor(out=s[:, :nb], in0=s[:, :nb],
                                               scalar=0.0, in1=em[:, :nb],
                                               op0=O.max, op1=O.add)
                # second elu on (y1-1)
                nc.vector.tensor_scalar_min(out=m[:, :nb], in0=s[:, :nb], scalar1=1.0)
                nc.scalar.activation(em[:, :nb], m[:, :nb], A.Exp, bias=negone[:])
                nc.vector.tensor_scalar_max(out=s[:, :nb], in0=s[:, :nb], scalar1=1.0)
                nc.vector.scalar_tensor_tensor(out=ost[:, :nb], in0=s[:, :nb],
                                               scalar=-2.0, in1=em[:, :nb],
                                               op0=O.add, op1=O.add)
            else:
                # mean over heads -> [*, nb, 32]
                h01 = fpool.tile([128, FB, 32], dt.float32, tag="h01")
                nc.vector.tensor_tensor(out=h01[:, :nb], in0=s[:, :nb, 0:32],
                                        in1=s[:, :nb, 32:64], op=O.add)
                nc.vector.tensor_tensor(out=ost[:, :nb], in0=s[:, :nb, 64:96],
                                        in1=s[:, :nb, 96:128], op=O.add)
                nc.vector.tensor_tensor(out=h01[:, :nb], in0=h01[:, :nb],
                                        in1=ost[:, :nb], op=O.add)
                nc.vector.tensor_scalar_mul(out=ost[:, :nb], in0=h01[:, :nb],
                                            scalar1=0.25)

            if nfull:
                nc.sync.dma_start(
                    out_d[r0:r0 + nfull * 128, :].rearrange("(c p) d -> p c d", p=128),
                    ost[:, :nfull, :])
            if tailr > 0:
                nc.sync.dma_start(out_d[r0 + nfull * 128:r1, :],
                                  ost[:tailr, nfull, :])

    nc.compile()
    return nc


# ---------------------------------------------------------------- runner ----
def make_runner(nc, n_cores, use_donate=False):
    install_neuronx_cc_hook()
    partition_name = nc.partition_id_tensor.name if nc.partition_id_tensor else None
    in_names, out_names, out_avals, zero_outs = [], [], [], []
    for alloc in nc.m.functions[0].allocations:
        if not isinstance(alloc, mybir.MemoryLocationSet):
            continue
        name = alloc.memorylocations[0].name
        if alloc.kind == "ExternalInput":
            if name != partition_name:
                in_names.append(name)
        elif alloc.kind == "ExternalOutput":
            dtp = mybir.dt.np(alloc.dtype)
            out_avals.append(jax.core.ShapedArray(tuple(alloc.tensor_shape), dtp))
            out_names.append(name)
            zero_outs.append(np.zeros(tuple(alloc.tensor_shape), dtp))
    n_params = len(in_names)
    n_outs = len(out_names)
    in_names.extend(out_names)
    if partition_name is not None:
        in_names.append(partition_name)
    donate = tuple(range(n_params, n_params + n_outs))

    def _body(*args):
        operands = list(args)
        if partition_name is not None:
            operands.append(bass2jax.partition_id_tensor())
        outs = _bass_exec_p.bind(
            *operands, out_avals=tuple(out_avals), in_names=tuple(in_names),
            out_names=tuple(out_names), lowering_input_output_aliases=(),
            sim_require_finite=True, sim_require_nnan=True, nc=nc)
        return tuple(outs)

    devices = jax.devices()[:n_cores]
    mesh = Mesh(np.asarray(devices), ("core",))
    sharded = jax.jit(
        shard_map(_body, mesh=mesh,
                  in_specs=(PartitionSpec("core"),) * (n_params + n_outs),
                  out_specs=(PartitionSpec("core"),) * n_outs,
                  check_rep=False),
        donate_argnums=(donate if use_donate else ()), keep_unused=True)

    class Runner:
        def __init__(self):
            self.in_names = in_names; self.out_names = out_names
            self.real_in_names = in_names[:n_params]
            self.out_avals = out_avals; self.n_cores = n_cores
        def prep(self, in_maps):
            concat = [np.concatenate([m[nm] for m in in_maps], axis=0) for nm in self.real_in_names]
            concat += [np.concatenate([z]*n_cores, axis=0) for z in zero_outs]
            sh = NamedSharding(mesh, PartitionSpec("core"))
            return [jax.device_put(a, sh) for a in concat]
        def run(self, dev_args):
            return sharded(*dev_args)
        def run_np(self, in_maps):
            outs = self.run(self.prep(in_maps))
            return [
                {nm: np.asarray(outs[i]).reshape(n_cores, *out_avals[i].shape)[c]
                 for i, nm in enumerate(out_names)}
                for c in range(n_cores)]
        def time_steady(self, dev_args, iters=6, warmup=2):
            for _ in range(warmup):
                jax.block_until_ready(self.run(dev_args))
            ts = []
            for _ in range(iters):
                t0 = time.perf_counter()
                jax.block_until_ready(self.run(dev_args))
                ts.append(time.perf_counter() - t0)
            return min(ts), ts
    return Runner()


# ------------------------------------------------------------- host glue ----
def layer_in_maps(plans, featsT_bf, feats_full, W_src, W_dst, b_src, b_dst,
                  attn, bias):
    """Build per-core in_maps for one layer launch."""
    bc_vals = (b_src + b_dst).astype(f32)
    # residual + out-bias + src-bias folded on host
    bias_fold = (bias.astype(f32) + b_src.astype(f32)).reshape(1, -1)
    attn_flat = attn.reshape(-1).astype(f32)          # [128]
    attnT = np.zeros((128, HEADS), f32)
    for h in range(HEADS):
        attnT[h * OUT:(h + 1) * OUT, h] = attn_flat[h * OUT:(h + 1) * OUT]
    attnT = attnT.astype(bf16)
    iota_rep = np.tile(np.arange(128, dtype=f32)[None, :], (128, 1)).astype(bf16)
    ident = np.eye(128, dtype=f32).astype(bf16)
    ones_row = np.ones((1, 128), bf16)
    bc_row = bc_vals.reshape(1, -1).astype(bf16)
    Ws = W_src.astype(bf16)
    Wd = W_dst.astype(bf16)
    in_maps = []
    for c, p in enumerate(plans):
        base = c * OWN
        def wrap16rep(a):
            return np.tile(a.reshape(-1, 16).T, (8, 1)).copy()
        in_maps.append(dict(
            featsT=featsT_bf,
            featsT_own=np.ascontiguousarray(featsT_bf[:, base:base + OWN]),
            feats_own=feats_full[base:base + OWN].astype(f32) + bias_fold,
            W_src=Ws, W_dst=Wd, bc_row=bc_row,
            idx_src=wrap16rep(p["idx_src"]), dstloc=p["dstloc"],
            attnT=attnT, iota_rep=iota_rep, ident=ident,
            ones_row=ones_row,
        ))
    return in_maps


class TwoLayerRunner:
    def __init__(self, src, dst, verbose=False):
        self.plans = make_plans(src, dst)
        self.T = self.plans[0]["T"]
        if verbose:
            print(f"common T={self.T} tiles ({self.T*128} idx slots)")
        self.nc0 = build_layer(self.plans[0], layer=0)
        self.nc1 = build_layer(self.plans[0], layer=1)
        self.r0 = make_runner(self.nc0, N_CORES)
        self.r1 = make_runner(self.nc1, N_CORES)

    def __call__(self, feats, inp):
        featsT_bf = np.ascontiguousarray(feats.T).astype(bf16)
        m0 = layer_in_maps(self.plans, featsT_bf, feats,
                           inp["W_src0"], inp["W_dst0"], inp["b_src0"][:],
                           inp["b_dst0"][:], inp["attn0"], inp["bias0"])
        outs0 = self.r0.run_np(m0)
        h1 = np.concatenate([o["out_own"] for o in outs0], axis=0)  # [N, 128]
        h1T_bf = np.ascontiguousarray(h1.T).astype(bf16)
        m1 = layer_in_maps(self.plans, h1T_bf, h1,
                           inp["W_src1"], inp["W_dst1"], inp["b_src1"][:],
                           inp["b_dst1"][:], inp["attn1"], inp["bias1"])
        outs1 = self.r1.run_np(m1)
        out = np.concatenate([o["out_own"] for o in outs1], axis=0)  # [N, 32]
        return h1, out


_TLR_CACHE = {}


def kernel(**inputs):
    inputs = {k: np.asarray(v) for k, v in inputs.items()}
    src = inputs["src"].astype(np.int64)
    dst = inputs["dst"].astype(np.int64)
    feats = inputs["feats"].astype(np.float32)
    kh = hash((src.tobytes(), dst.tobytes()))
    if kh not in _TLR_CACHE:
        _TLR_CACHE[kh] = TwoLayerRunner(src, dst)
    tlr = _TLR_CACHE[kh]
    _h1, out = tlr(feats, inputs)
    return out.astype(np.float32)


_NULL_CACHE = {}


def null_baseline():
    """Steady-state wall of a near-empty 8-core launch (dispatch overhead)."""
    if "t" in _NULL_CACHE:
        return _NULL_CACHE["t"]
    nc = bacc.Bacc("TRN2", target_bir_lowering=False, debug=False,
                   num_devices=N_CORES)
    x = nc.dram_tensor("x", (128, 128), dt.float32, kind="ExternalInput")
    y = nc.dram_tensor("y", (128, 128), dt.float32, kind="ExternalOutput")
    with tile.TileContext(nc) as tc, ExitStack() as ctx:
        pool = ctx.enter_context(tc.tile_pool(name="sbuf", bufs=2))
        t = pool.tile([128, 128], dt.float32)
        nc.sync.dma_start(t[:], x[:])
        nc.sync.dma_start(y[:], t[:])
    nc.compile()
    r = make_runner(nc, N_CORES)
    xs = np.zeros((128, 128), np.float32)
    dev = r.prep([{"x": xs}] * N_CORES)
    best, _ = r.time_steady(dev, iters=8, warmup=2)
    _NULL_CACHE["t"] = best
    return best
